# revision 1
# baseline (speedup 1.0000x reference)
"""Multi-head attention (B=8, N=1024, C=1024, H=16) on 8 TRN2 NeuronCores.

Strategy: pure data parallelism — one batch element per core, no collectives.
Layouts avoid all on-device transposes:

  host passes    xT = x[b].T              [C, N]   (c on partitions)
                 wT = qkv_w.T             [C, 3C]  (q-columns pre-scaled)
                 pT = proj_w.T            [C, C]
  device makes   V         [n, d] natural layout, with a ones column per head
                 Q^T, K^T  [d, n] computed per head-pair (rotating pool)
                 S^T = lhsT(K^T_h) x Q^T_h   [n_k, n_q]  — the two heads of a
                     pair run CONCURRENTLY in the PE array via tile_position
                     row packing (K=64 each, rows 0-63 / 64-127)
                 P^T = exp(S^T)           (no max-subtract: |S|<=~10, safe)
                 O'^T = [V_h|1].T @ P^T   [65, n_q]  (row 64 = softmax denom Z)
                 O^T  = O'^T[0:64] * (1/Z)  (reciprocal_approx_fast; 1/Z
                     broadcast across partitions via a DRAM-bounce DMA)
                 y^T = pT.T @ O^T + b     [C, N]
  host returns   y = yT.T  per batch.

All matmuls are float32r (full-rate fp32). The per-pair software pipeline
(qk projection of pair j+1 emitted between the AV stages of pair j) keeps
the PE dense with full-array work while ACT runs the exp chains.
"""

import contextlib

import numpy as np

import concourse.bass as bass
import concourse.mybir as mybir
import concourse.tile as tile
from concourse import bacc
from concourse.bass_utils import run_bass_kernel_spmd

f32 = mybir.dt.float32
f32r = mybir.dt.float32r
EXP = mybir.ActivationFunctionType.Exp

B, N, C = 8, 1024, 1024
H, HD = 16, 64
SCALE = HD ** -0.5
NCORES = 8


def mm(nc, out, lhsT, rhs, start, stop, tile_position=None):
    nc.tensor.matmul(out, lhsT, rhs, start=start, stop=stop,
                     tile_position=tile_position)


def _rep(tc, reps):
    if reps <= 1:
        return contextlib.nullcontext()
    return tc.For_i(0, reps, 1, hint_engines=(
        mybir.EngineType.PE, mybir.EngineType.Activation,
        mybir.EngineType.DVE, mybir.EngineType.SP, mybir.EngineType.Pool))


def build(stages="ABC", reps=1):
    nc = bacc.Bacc("TRN2", target_bir_lowering=False, debug=False)
    xT = nc.dram_tensor("xT", [C, N], f32, kind="ExternalInput")
    wT = nc.dram_tensor("wT", [C, 3 * C], f32, kind="ExternalInput")
    pT = nc.dram_tensor("pT", [C, C], f32, kind="ExternalInput")
    pb = nc.dram_tensor("pb", [C, 1], f32, kind="ExternalInput")
    yT = nc.dram_tensor("yT", [C, N], f32, kind="ExternalOutput")

    with tile.TileContext(nc) as tc:
        with (
            tc.tile_pool(name="const", bufs=1) as const,
            tc.tile_pool(name="xp", bufs=8) as xp,
            tc.tile_pool(name="vpp", bufs=8) as vpp,
            tc.tile_pool(name="obp", bufs=8) as obp,
            tc.tile_pool(name="qkp", bufs=4) as qkp,
            tc.tile_pool(name="wqkp", bufs=4) as wqkp,
            tc.tile_pool(name="psQ", bufs=1, space="PSUM") as psQ,
        ):
            onesc = const.tile([128, H, 1], f32)
            nc.vector.memset(onesc, 1.0)

            xts = [xp.tile([128, N], f32r, name=f"xt{i}", tag="xt")
                   for i in range(8)]
            for ci in range(8 if stages else 0):
                # xts[0] goes on the sync queue right before the first
                # weight DMA; the rest stream in parallel on the gpsimd
                # (SWDGE) queues so the first matmul starts ~1MB of DMA in
                eng = nc.sync if ci == 0 else nc.gpsimd
                eng.dma_start(
                    out=xts[ci],
                    in_=xT[ci * 128:(ci + 1) * 128, :].bitcast(f32r))

            # V' resident: [n-chunk][128, 16 heads, 64+1]; col 64 is ones.
            vp = [vpp.tile([128, H, HD + 1], f32r, name=f"vp{i}", tag="vp")
                  for i in range(8)]
            # O^T resident: tile j = rows [j*128,(j+1)*128) = heads 2j,2j+1
            ob = [obp.tile([128, N], f32r, name=f"ob{i}", tag="ob")
                  for i in range(8)]

            def qk_chunk(d, jname):
                """Project wT columns [d*128,(d+1)*128) -> [128, N]."""
                wt = wqkp.tile([128, 8, 128], f32r,
                               name=f"wt{jname}", tag="wt")
                nc.sync.dma_start(
                    out=wt,
                    in_=wT[:, d * 128:(d + 1) * 128]
                    .rearrange("(j p) c -> p j c", p=128).bitcast(f32r))
                acc = psQ.tile([128, N], f32, name="qacc", tag="qacc")
                for qh in range(2):
                    sl = slice(qh * 512, (qh + 1) * 512)
                    for ci in range(8):
                        mm(nc, acc[:, sl], wt[:, ci, :],
                           xts[ci][:, sl],
                           start=(ci == 0), stop=(ci == 7))
                qt = qkp.tile([128, N], f32r, name=f"qk{jname}", tag="qk")
                nc.vector.tensor_copy(qt[:, :], acc[:, :])
                return qt

            def qs_stage(j):
                QT = qk_chunk(j, f"q{j}")
                KT = qk_chunk(8 + j, f"k{j}")
                return QT, KT

            # prefetch pair 0's q/k projection ahead of the V phase so the
            # PE rolls straight from V matmuls into attention
            qks = qs_stage(0) if "B" in stages else None

            # ---------------- phase V: value projection ----------------
            with (
                tc.tile_pool(name="wvp", bufs=2) as wvp,
                tc.tile_pool(name="psV", bufs=3, space="PSUM") as psV,
            ):
                with _rep(tc, reps):
                    for dv in range(2 if "A" in stages else 0):
                        wv = wvp.tile([128, 8, 512], f32r, name="wv",
                                      tag="wv")
                        nc.sync.dma_start(
                            out=wv,
                            in_=wT[:, 2048 + dv * 512:2048 + (dv + 1) * 512]
                            .rearrange("(j p) c -> p j c", p=128)
                            .bitcast(f32r))
                        for n in range(8):
                            vacc = psV.tile([128, 512], f32, name="vacc",
                                            tag="vacc")
                            for ci in range(8):
                                mm(nc, vacc[:, :],
                                   xts[ci][:, n * 128:(n + 1) * 128],
                                   wv[:, ci, :],
                                   start=(ci == 0), stop=(ci == 7))
                            nc.vector.tensor_copy(
                                vp[n][:, dv * 8:(dv + 1) * 8, 0:HD],
                                vacc.rearrange("p (g e) -> p g e", e=HD))
                    for n in range(8 if "A" in stages else 0):
                        nc.vector.tensor_copy(vp[n][:, :, HD:HD + 1],
                                              onesc[:, :, :])

            # stage-C weights/bias: allocated here so their DMAs can
            # prefetch during the attention pairs
            wpp_ctx = tc.tile_pool(name="wpp", bufs=2)
            biasp_ctx = tc.tile_pool(name="biasp", bufs=8)
            wpp = wpp_ctx.__enter__()
            biasp = biasp_ctx.__enter__()
            nC = 8 if "C" in stages else 0
            pbt = [biasp.tile([128, 1], f32, name=f"pbt{e}", tag="pbt")
                   for e in range(8)]
            for e in range(nC):
                nc.sync.dma_start(out=pbt[e],
                                  in_=pb[e * 128:(e + 1) * 128, :])

            def load_wpt(e):
                wpt = wpp.tile([128, 8, 128], f32r, name="wpt", tag="wpt")
                nc.sync.dma_start(
                    out=wpt,
                    in_=pT[:, e * 128:(e + 1) * 128]
                    .rearrange("(j p) c -> p j c", p=128).bitcast(f32r))
                return wpt

            wpts = {e: load_wpt(e) for e in range(min(nC, 2))}

            # ---------------- attention pairs (fused qk-proj + attn) -----
            with (
                tc.tile_pool(name="ppool", bufs=10) as ppool,
                tc.tile_pool(name="ovsp", bufs=2) as ovsp,
                tc.tile_pool(name="rbsp", bufs=2) as rbsp,
                tc.tile_pool(name="otp", bufs=1) as otp,
                tc.tile_pool(name="yp", bufs=2) as yp,
                tc.tile_pool(name="psSA", bufs=1, space="PSUM") as psSA,
                tc.tile_pool(name="psSB", bufs=1, space="PSUM") as psSB,
                tc.tile_pool(name="psO", bufs=2, space="PSUM") as psO,
                tc.tile_pool(name="rdr", bufs=4, space="DRAM") as rdr,
            ):
                def s_stage(j, QT, KT):
                    """Packed S^T for heads 2j (rows 0-63) and 2j+1
                    (rows 64-127): both run concurrently in the array."""
                    ptsA = [ppool.tile([128, N], f32r,
                                       name=f"pa{j}_{kc}", tag="pt")
                            for kc in range(8)]
                    ptsB = [ppool.tile([128, N], f32r,
                                       name=f"pb{j}_{kc}", tag="pt")
                            for kc in range(8)]
                    for kc in range(8):
                        stA = psSA.tile([128, N], f32, name="stA",
                                        tag="stA")
                        stB = psSB.tile([128, N], f32, name="stB",
                                        tag="stB")
                        ks = slice(kc * 128, (kc + 1) * 128)
                        for qh in range(2):
                            sl = slice(qh * 512, (qh + 1) * 512)
                            mm(nc, stA[:, sl], KT[0:64, ks],
                               QT[0:64, sl], start=True, stop=True,
                               tile_position=(0, 0))
                            mm(nc, stB[:, sl], KT[64:128, ks],
                               QT[64:128, sl], start=True, stop=True,
                               tile_position=(64, 0))
                        nc.scalar.activation(ptsA[kc][:, :], stA[:, :], EXP)
                        nc.scalar.activation(ptsB[kc][:, :], stB[:, :], EXP)
                    return ptsA, ptsB

                def av_stage(h, pts):
                    hq, ho = h // 2, (h % 2) * 64
                    ov = [psO.tile([65, 512], f32,
                                   name=f"ov{h}_{qh}", tag="ov")
                          for qh in range(2)]
                    for kc in range(8):
                        for qh in range(2):
                            sl = slice(qh * 512, (qh + 1) * 512)
                            mm(nc, ov[qh][:, :], vp[kc][:, h, :],
                               pts[kc][:, sl],
                               start=(kc == 0), stop=(kc == 7))
                    # copy O'^T out of PSUM immediately so the ov slots
                    # free for the next head's AV; then 1/Z in place on the
                    # SBUF copy and broadcast via a DRAM bounce (step-0 read)
                    ovs = ovsp.tile([65, N], f32, name=f"ovs{h}", tag="ovs")
                    for qh in range(2):
                        sl = slice(qh * 512, (qh + 1) * 512)
                        nc.vector.tensor_copy(ovs[:, sl], ov[qh][:, :])
                    rbs = rbsp.tile([64, N], f32, name="rbs", tag="rbs")
                    for qh in range(2):
                        sl = slice(qh * 512, (qh + 1) * 512)
                        nc.vector.reciprocal(ovs[64:65, sl],
                                             ovs[64:65, sl])
                        rsc = rdr.tile([1, 512], f32, name="rsc", tag="rsc")
                        nc.sync.dma_start(out=rsc, in_=ovs[64:65, sl])
                        bsrc = bass.AP(tensor=rsc.tensor, offset=rsc.offset,
                                       ap=[[0, 64], [1, 512]])
                        nc.sync.dma_start(out=rbs[:, sl], in_=bsrc)
                    if ho == 0:
                        for qh in range(2):
                            sl = slice(qh * 512, (qh + 1) * 512)
                            nc.vector.tensor_mul(ob[hq][0:64, sl],
                                                 ovs[0:64, sl],
                                                 rbs[:, sl])
                    else:
                        ot = otp.tile([64, N], f32r, name="ot", tag="ot")
                        for qh in range(2):
                            sl = slice(qh * 512, (qh + 1) * 512)
                            nc.vector.tensor_mul(ot[:, sl],
                                                 ovs[0:64, sl],
                                                 rbs[:, sl])
                        # partition shift 0-63 -> 64-127 needs a DMA
                        nc.gpsimd.dma_start(out=ob[hq][64:128, :],
                                            in_=ot[:, :])

                def proj_head(e):
                    """Stage-C chunk e, d=0..6 partial accumulation (these
                    only read ob tiles finished by earlier pairs)."""
                    wpt = wpts.pop(e, None)
                    if wpt is None:
                        wpt = load_wpt(e)
                    pool_, tag_ = ((psQ, "qacc") if e % 2 == 0
                                   else (psSA, "stA"))
                    pj = pool_.tile([128, N], f32, name="pj", tag=tag_)
                    for qh in range(2):
                        sl = slice(qh * 512, (qh + 1) * 512)
                        for d in range(7):
                            mm(nc, pj[:, sl], wpt[:, d, :],
                               ob[d][:, sl],
                               start=(d == 0), stop=False)
                    return pj, wpt

                def proj_tail(e, pj, wpt):
                    for qh in range(2):
                        sl = slice(qh * 512, (qh + 1) * 512)
                        mm(nc, pj[:, sl], wpt[:, 7, :], ob[7][:, sl],
                           start=False, stop=True)
                    yt = yp.tile([128, N], f32, name="yt", tag="yt")
                    nc.vector.tensor_scalar_add(yt[:, :], pj[:, :],
                                                pbt[e])
                    nc.sync.dma_start(
                        out=yT[e * 128:(e + 1) * 128, :], in_=yt[:, :])

                def do_proj(e):
                    proj_tail(e, *proj_head(e))

                with _rep(tc, reps):
                    npairs = 8 if "B" in stages else 0
                    pend = None
                    for j in range(npairs):
                        pj = s_stage(j, *qks)
                        if j + 1 < npairs:
                            qks = qs_stage(j + 1)
                        if pend is not None:
                            av_stage(2 * pend[0], pend[1][0])
                            av_stage(2 * pend[0] + 1, pend[1][1])
                        pend = (j, pj)
                    if pend is not None:
                        av_stage(2 * pend[0], pend[1][0])
                        # first proj chunk's d=0..6 partials emitted between
                        # the final AV stages: they fill the PE while the
                        # last exp/normalize tail drains; d=7 completes after
                        head0 = proj_head(0) if nC else None
                        av_stage(2 * pend[0] + 1, pend[1][1])
                        if head0 is not None:
                            proj_tail(0, *head0)

                # ---------- stage C: output projection (same context, so
                # no pool-release barrier between attention and proj) ----
                with _rep(tc, reps):
                    for e in range(1 if ("B" in stages and npairs) else 0,
                                   nC):
                        do_proj(e)
            biasp_ctx.__exit__(None, None, None)
            wpp_ctx.__exit__(None, None, None)
    nc.compile()
    return nc


_CACHE = {}


def _get_nc():
    if "nc" not in _CACHE:
        _CACHE["nc"] = build()
    return _CACHE["nc"]


def _prep_in_maps(x, qkv_w, proj_w, proj_b):
    wT = np.ascontiguousarray(qkv_w.T).astype(np.float32)
    wT[:, 0:C] *= np.float32(SCALE)
    pT = np.ascontiguousarray(proj_w.T).astype(np.float32)
    pbv = np.ascontiguousarray(np.asarray(proj_b, dtype=np.float32)
                               .reshape(C, 1))
    return [
        {
            "xT": np.ascontiguousarray(np.asarray(x[b], dtype=np.float32).T),
            "wT": wT,
            "pT": pT,
            "pb": pbv,
        }
        for b in range(B)
    ]


def kernel(x, qkv_w, proj_w, proj_b):
    x = np.asarray(x)
    assert x.shape == (B, N, C), x.shape
    nc = _get_nc()
    in_maps = _prep_in_maps(x, qkv_w, proj_w, proj_b)
    res = run_bass_kernel_spmd(nc, in_maps, core_ids=list(range(NCORES)))
    out = np.stack([res.results[b]["yT"].T for b in range(B)], axis=0)
    return np.ascontiguousarray(out.astype(np.float32))



# revision 6
# speedup vs baseline: 1.2276x; 1.2276x over previous
"""Multi-head attention (B=8, N=1024, C=1024, H=16) on 8 TRN2 NeuronCores.

Strategy: pure data parallelism — one batch element per core, no collectives.
All matmul operands are bf16 (PSUM accumulation stays fp32): fp32r streams
slower on HW and power-throttles the PE; bf16 rel-err through this network
is ~0.5-1%, well inside the 2e-2 gate.

Layouts avoid all on-device transposes AND all partition-shift DMAs:

  host passes    xT = x[b].T            [C, N]  bf16 (c on partitions)
                 wT = qkv_w.T           [C, 3C] bf16 (q cols pre-scaled,
                     v cols permuted even-heads-first per 8-head group)
                 pT = proj_w.T          [C, C]  bf16
  device makes   V' resident per n-chunk: [128, 8 pairs, 129] with per-pair
                     cols [V_even(0:64) | ones(64) | V_odd(65:129)]
                 Q^T, K^T [d, n] per head-pair (chunks of 128 C-rows)
                 S^T packed [128, 2N] per key-chunk: heads 2j / 2j+1 run
                     CONCURRENTLY in the PE array via tile_position row
                     packing (K=64 each, rows 0-63 / 64-127)
                 P^T = exp(S^T) bf16    (no max-subtract: |S|<=~8, safe)
                 O'^T: even head lhsT=[V_e|1] -> rows 0-63 + Z at row 64;
                     odd head lhsT=[.|1|V_o] (128 cols) -> rows 64-127 + Z
                     at row 63 — odd heads land directly on partitions
                     64-127, so no partition-shift DMA is ever needed
                 Z broadcast via gpsimd partition_broadcast, reciprocal on
                     the [64, N] broadcast tile (128-lane DVE, not 1-lane)
                 y^T = pT.T @ O^T + b   [C, N] fp32
  host returns   y = yT.T per batch.

The per-pair software pipeline (qk projection of pair j+1 emitted between
the S and AV stages of pair j; V-value projection emitted after pair 0's
S stage so exp runs while the PE does V matmuls; proj partials interleaved
between the final AV stages) keeps PE and ACT both near-saturated.
"""

import numpy as np
import ml_dtypes

import concourse.bass as bass  # noqa: F401  (AP construction if needed)
import concourse.mybir as mybir
import concourse.tile as tile
from concourse import bacc
from concourse.bass_utils import run_bass_kernel_spmd

f32 = mybir.dt.float32
bf16 = mybir.dt.bfloat16
EXP = mybir.ActivationFunctionType.Exp

B, N, C = 8, 1024, 1024
H, HD = 16, 64
SCALE = HD ** -0.5
NCORES = 8


def mm(nc, out, lhsT, rhs, start, stop, tile_position=None):
    nc.tensor.matmul(out, lhsT, rhs, start=start, stop=stop,
                     tile_position=tile_position)


def build():
    nc = bacc.Bacc("TRN2", target_bir_lowering=False, debug=False)
    xT = nc.dram_tensor("xT", [C, N], bf16, kind="ExternalInput")
    wT = nc.dram_tensor("wT", [C, 3 * C], bf16, kind="ExternalInput")
    pT = nc.dram_tensor("pT", [C, C], bf16, kind="ExternalInput")
    pb = nc.dram_tensor("pb", [128, 8], f32, kind="ExternalInput")
    yT = nc.dram_tensor("yT", [C, N], f32, kind="ExternalOutput")

    with tile.TileContext(nc) as tc:
        with (
            tc.tile_pool(name="xp", bufs=8) as xp,
            tc.tile_pool(name="vpp", bufs=8) as vpp,
            tc.tile_pool(name="obp", bufs=8) as obp,
            tc.tile_pool(name="qkp", bufs=4) as qkp,
            tc.tile_pool(name="wqkp", bufs=4) as wqkp,
            tc.tile_pool(name="wvp", bufs=2) as wvp,
            tc.tile_pool(name="wpp", bufs=8) as wpp,
            tc.tile_pool(name="biasp", bufs=1) as biasp,
            tc.tile_pool(name="ppool", bufs=16) as ppool,
            tc.tile_pool(name="ovsp", bufs=2) as ovsp,
            tc.tile_pool(name="rbsp", bufs=2) as rbsp,
            tc.tile_pool(name="yp", bufs=2) as yp,
            tc.tile_pool(name="rdr", bufs=4, space="DRAM") as rdr,
            tc.tile_pool(name="psQ", bufs=1, space="PSUM") as psQ,
            tc.tile_pool(name="psS", bufs=1, space="PSUM") as psS,
            tc.tile_pool(name="psO", bufs=2, space="PSUM") as psO,
        ):
            def load_wt(d):
                wt = wqkp.tile([128, 8, 128], bf16, name=f"wt{d}", tag="wt")
                nc.sync.dma_start(
                    out=wt,
                    in_=wT[:, d * 128:(d + 1) * 128]
                    .rearrange("(j p) c -> p j c", p=128))
                return wt

            # q0's weights lead the sync queue so matmul 1 starts early
            wtq0 = load_wt(0)

            xts = [xp.tile([128, N], bf16, name=f"xt{i}", tag="xt")
                   for i in range(8)]
            for ci in range(8):
                eng = nc.sync if ci == 0 else nc.gpsimd
                eng.dma_start(out=xts[ci],
                              in_=xT[ci * 128:(ci + 1) * 128, :])

            wtk0 = load_wt(8)

            def qk_compute(wt, jname):
                acc = psQ.tile([128, N], f32, name="qacc", tag="qacc")
                for qh in range(2):
                    sl = slice(qh * 512, (qh + 1) * 512)
                    for ci in range(8):
                        mm(nc, acc[:, sl], wt[:, ci, :], xts[ci][:, sl],
                           start=(ci == 0), stop=(ci == 7))
                qt = qkp.tile([128, N], bf16, name=f"qk{jname}", tag="qk")
                nc.vector.tensor_copy(qt[:, :], acc[:, :])
                return qt

            def qs_stage(j, pre=None):
                wq = pre[0] if pre else load_wt(j)
                wk = pre[1] if pre else load_wt(8 + j)
                return qk_compute(wq, f"q{j}"), qk_compute(wk, f"k{j}")

            qks = qs_stage(0, pre=(wtq0, wtk0))

            # V' resident: [n][128, pair, 160] =
            # [V_even(0:64) | ones(64:96) | V_odd(96:160)].  The odd head's
            # lhsT window [32:160] puts a ones column at window col 32, so
            # its Z lands on partition 32 (engine reads must start at a
            # 32-partition boundary) while O lands on partitions 64-127.
            VW = 160
            vp = [vpp.tile([128, 8, VW], bf16, name=f"vp{i}",
                           tag="vp") for i in range(8)]
            # O^T resident: tile j = rows [j*128,(j+1)*128) = heads 2j,2j+1
            ob = [obp.tile([128, N], bf16, name=f"ob{i}", tag="ob")
                  for i in range(8)]

            def s_stage(j, QT, KT):
                """Packed S^T -> P^T for heads 2j (cols 0:N) and 2j+1
                (cols N:2N); one exp per key-chunk."""
                pts = []
                for kc in range(8):
                    st = psS.tile([128, 2 * N], f32, name=f"st{j}_{kc}",
                                  tag="st")
                    ks = slice(kc * 128, (kc + 1) * 128)
                    for qh in range(2):
                        sl = slice(qh * 512, (qh + 1) * 512)
                        mm(nc, st[:, sl], KT[0:64, ks], QT[0:64, sl],
                           start=True, stop=True, tile_position=(0, 0))
                        mm(nc, st[:, N + qh * 512:N + (qh + 1) * 512],
                           KT[64:128, ks], QT[64:128, sl],
                           start=True, stop=True, tile_position=(64, 0))
                    pt = ppool.tile([128, 2 * N], bf16, name=f"pt{j}_{kc}",
                                    tag="pt")
                    nc.scalar.activation(pt[:, :], st[:, :], EXP)
                    pts.append(pt)
                return pts

            # pair 0's S stage first: ACT starts exp'ing while the PE
            # runs the V projection below
            pts0 = s_stage(0, *qks)

            # ---------------- value projection (vacc in the psO ring) ----
            wvs = []
            for dv in range(2):
                wv = wvp.tile([128, 8, 512], bf16, name=f"wv{dv}", tag="wv")
                nc.gpsimd.dma_start(
                    out=wv,
                    in_=wT[:, 2048 + dv * 512:2048 + (dv + 1) * 512]
                    .rearrange("(j p) c -> p j c", p=128))
                wvs.append(wv)
            for dv in range(2):
                for n in range(8):
                    vacc = psO.tile([128, 512], f32, name="vacc", tag="ov")
                    for ci in range(8):
                        mm(nc, vacc[:, :], xts[ci][:, n * 128:(n + 1) * 128],
                           wvs[dv][:, ci, :],
                           start=(ci == 0), stop=(ci == 7))
                    # host permuted v cols: [even-heads(256) | odd(256)]
                    ps = slice(dv * 4, (dv + 1) * 4)
                    nc.vector.tensor_copy(
                        vp[n][:, ps, 0:HD],
                        vacc[:, 0:256].rearrange("p (g e) -> p g e", e=HD))
                    nc.vector.tensor_copy(
                        vp[n][:, ps, 96:160],
                        vacc[:, 256:512].rearrange("p (g e) -> p g e", e=HD))
            for n in range(8):
                nc.vector.memset(vp[n][:, :, HD:96], 1.0)

            # stage-C prefetch (idle gpsimd queue during attention)
            pbt = biasp.tile([128, 8], f32, name="pbt", tag="pbt")
            nc.gpsimd.dma_start(out=pbt, in_=pb[:, :])

            def load_wpt(e):
                wpt = wpp.tile([128, 8, 128], bf16, name=f"wpt{e}",
                               tag="wpt")
                nc.gpsimd.dma_start(
                    out=wpt,
                    in_=pT[:, e * 128:(e + 1) * 128]
                    .rearrange("(j p) c -> p j c", p=128))
                return wpt

            wpts = [load_wpt(e) for e in range(8)]

            def av_stage(h, pts):
                hq, odd = h // 2, h % 2
                off = odd * N
                ov = [psO.tile([128, 512], f32, name=f"ov{h}_{qh}",
                               tag="ov") for qh in range(2)]
                for kc in range(8):
                    lhsT = (vp[kc][:, hq, 32:160] if odd
                            else vp[kc][:, hq, 0:HD + 1])
                    for qh in range(2):
                        sl = slice(off + qh * 512, off + (qh + 1) * 512)
                        outap = ov[qh][:, :] if odd else ov[qh][0:HD + 1, :]
                        mm(nc, outap, lhsT, pts[kc][:, sl],
                           start=(kc == 0), stop=(kc == 7))
                # copy O'^T (+ Z row) out of PSUM immediately so the ov
                # slots free for the next head's AV
                ovs = ovsp.tile([128, N], f32, name=f"ovs{h}", tag="ovs")
                for qh in range(2):
                    sl = slice(qh * 512, (qh + 1) * 512)
                    if odd:
                        nc.vector.tensor_copy(ovs[32:33, sl],
                                              ov[qh][32:33, :])
                        nc.vector.tensor_copy(ovs[64:128, sl],
                                              ov[qh][64:128, :])
                    else:
                        nc.vector.tensor_copy(ovs[0:HD + 1, sl],
                                              ov[qh][0:HD + 1, :])
                r0 = 64 * odd
                zr = 32 if odd else 64
                rbs = rbsp.tile([128, N], f32, name=f"rbs{h}", tag="rbs")
                rsc = rdr.tile([1, N], f32, name=f"rsc{h}", tag="rsc")
                nc.gpsimd.dma_start(out=rsc, in_=ovs[zr:zr + 1, :])
                bsrc = bass.AP(tensor=rsc.tensor, offset=rsc.offset,
                               ap=[[0, 64], [1, N]])
                nc.gpsimd.dma_start(out=rbs[r0:r0 + 64, :], in_=bsrc)
                nc.vector.reciprocal(rbs[r0:r0 + 64, :], rbs[r0:r0 + 64, :])
                nc.vector.tensor_mul(ob[hq][r0:r0 + 64, :],
                                     ovs[r0:r0 + 64, :], rbs[r0:r0 + 64, :])

            def proj_head(e):
                """Proj chunk e, d=0..6 partial accumulation (reads ob
                tiles finished by earlier pairs)."""
                pool_, tag_ = (psQ, "qacc") if e % 2 == 0 else (psS, "st")
                pj = pool_.tile([128, N], f32, name=f"pj{e}", tag=tag_)
                for qh in range(2):
                    sl = slice(qh * 512, (qh + 1) * 512)
                    for d in range(7):
                        mm(nc, pj[:, sl], wpts[e][:, d, :], ob[d][:, sl],
                           start=(d == 0), stop=False)
                return pj

            def proj_tail(e, pj):
                yt = yp.tile([128, N], f32, name=f"yt{e}", tag="yt")
                for qh in range(2):
                    sl = slice(qh * 512, (qh + 1) * 512)
                    mm(nc, pj[:, sl], wpts[e][:, 7, :], ob[7][:, sl],
                       start=False, stop=True)
                for qh in range(2):
                    sl = slice(qh * 512, (qh + 1) * 512)
                    nc.vector.tensor_scalar_add(yt[:, sl], pj[:, sl],
                                                pbt[:, e:e + 1])
                    nc.sync.dma_start(out=yT[e * 128:(e + 1) * 128, sl],
                                      in_=yt[:, sl])

            # ---------------- attention pairs (fused qk-proj + attn) -----
            pend = (0, pts0)
            for j in range(1, 8):
                pts = s_stage(j, *qs_stage(j))
                av_stage(2 * pend[0], pend[1])
                av_stage(2 * pend[0] + 1, pend[1])
                pend = (j, pts)
            # tail: proj partials interleaved between the final AV stages
            av_stage(15, pend[1])
            pj0 = proj_head(0)
            av_stage(14, pend[1])
            pj1 = proj_head(1)
            proj_tail(0, pj0)
            pj2 = proj_head(2)
            proj_tail(1, pj1)
            proj_tail(2, pj2)
            for e in range(3, 8):
                proj_tail(e, proj_head(e))
    nc.compile()
    return nc


_CACHE = {}


def _get_nc():
    if "nc" not in _CACHE:
        _CACHE["nc"] = build()
    return _CACHE["nc"]


def _prep_in_maps(x, qkv_w, proj_w, proj_b):
    w = np.asarray(qkv_w, dtype=np.float32).copy()
    w[0:C, :] *= np.float32(SCALE)  # fold the attention scale into Wq
    # permute v output cols per 8-head group: even heads first, so the
    # device's V' copies are two contiguous strided views
    perm = []
    for dv in range(2):
        base = 2 * C + dv * 512
        for hh in (0, 2, 4, 6, 1, 3, 5, 7):
            perm.extend(range(base + hh * 64, base + (hh + 1) * 64))
    w[2 * C:3 * C, :] = w[perm, :]
    wT = np.ascontiguousarray(w.T).astype(ml_dtypes.bfloat16)
    pT = np.ascontiguousarray(
        np.asarray(proj_w, dtype=np.float32).T).astype(ml_dtypes.bfloat16)
    pbv = np.ascontiguousarray(
        np.asarray(proj_b, dtype=np.float32).reshape(8, 128).T)
    return [
        {
            "xT": np.ascontiguousarray(
                np.asarray(x[b], dtype=np.float32).T
            ).astype(ml_dtypes.bfloat16),
            "wT": wT,
            "pT": pT,
            "pb": pbv,
        }
        for b in range(B)
    ]


def kernel(x, qkv_w, proj_w, proj_b):
    x = np.asarray(x)
    assert x.shape == (B, N, C), x.shape
    nc = _get_nc()
    in_maps = _prep_in_maps(x, qkv_w, proj_w, proj_b)
    res = run_bass_kernel_spmd(nc, in_maps, core_ids=list(range(NCORES)))
    out = np.stack([res.results[b]["yT"].T for b in range(B)], axis=0)
    return np.ascontiguousarray(out.astype(np.float32))
